# revision 2
# baseline (speedup 1.0000x reference)
import numpy as np
import ml_dtypes

B, H, N, D = 4, 12, 8192, 64
M = 128
NCORES = 8
PAIRS = (B * H) // NCORES
NCHUNK = 512
NCH = N // NCHUNK
NT = N // 128

_cache = {}


def _build():
    if "nc" in _cache:
        return _cache["nc"]
    import concourse.bacc as bacc
    import concourse.mybir as mybir
    import concourse.tile as tile

    f32, f32r, bf16 = mybir.dt.float32, mybir.dt.float32r, mybir.dt.bfloat16
    AF = mybir.ActivationFunctionType

    nc = bacc.Bacc("TRN2", target_bir_lowering=False, debug=False)
    QT = nc.declare_dram_parameter("QT", [PAIRS, 64, N], f32, isOutput=False)
    KT = nc.declare_dram_parameter("KT", [PAIRS, 64, N], f32, isOutput=False)
    Vb = nc.declare_dram_parameter("Vb", [PAIRS, N, 64], bf16, isOutput=False)
    NCT = nc.declare_dram_parameter("NCT", [PAIRS, 64, M], f32, isOutput=False)
    NRT = nc.declare_dram_parameter("NRT", [PAIRS, 64, M], f32, isOutput=False)
    GS = nc.declare_dram_parameter("GS", [1, 1], f32, isOutput=False)
    XO = nc.declare_dram_parameter("XO", [PAIRS, N, 64], f32, isOutput=True)

    with tile.TileContext(nc) as tc:
        with (tc.tile_pool(name="p", bufs=1) as pool,
              tc.tile_pool(name="pd", bufs=2) as poold,
              tc.tile_pool(name="ps", bufs=2, space="PSUM") as psum,
              tc.tile_pool(name="pss", bufs=1, space="PSUM") as pss):
            ident_bf = pool.tile([128, 128], bf16, tag="ident")
            nc.gpsimd.memset(ident_bf[:], 0.0)
            nc.gpsimd.affine_select(out=ident_bf[:], in_=ident_bf[:],
                compare_op=mybir.AluOpType.not_equal, fill=1.0, base=0,
                pattern=[[-1, 128]], channel_multiplier=1)
            i7 = pool.tile([128, 128], bf16, tag="i7")
            nc.gpsimd.memset(i7[:], 0.0)
            nc.gpsimd.affine_select(out=i7[:], in_=i7[:],
                compare_op=mybir.AluOpType.not_equal, fill=7.0, base=0,
                pattern=[[-1, 128]], channel_multiplier=1)
            i15 = pool.tile([128, 128], bf16, tag="i15")
            nc.gpsimd.memset(i15[:], 0.0)
            nc.gpsimd.affine_select(out=i15[:], in_=i15[:],
                compare_op=mybir.AluOpType.not_equal, fill=15.0, base=0,
                pattern=[[-1, 128]], channel_multiplier=1)
            i13 = pool.tile([128, 128], bf16, tag="i13")
            nc.gpsimd.memset(i13[:], 0.0)
            nc.gpsimd.affine_select(out=i13[:], in_=i13[:],
                compare_op=mybir.AluOpType.not_equal, fill=13.0, base=0,
                pattern=[[-1, 128]], channel_multiplier=1)
            ones_row = pool.tile([1, 128], f32, tag="ones_row")
            nc.vector.memset(ones_row[:], 1.0)
            gs_sb = pool.tile([1, 1], f32, tag="gs_sb")
            nc.sync.dma_start(gs_sb[:], GS[:])
            ps_bc = pss.tile([128, 1], f32, tag="ps_bc")
            nc.tensor.matmul(ps_bc[:], ones_row[:], gs_sb[:], start=True, stop=True)
            gsb = pool.tile([128, 1], f32, tag="gsb")
            nc.scalar.copy(gsb[:], ps_bc[:])

            for p in range(PAIRS):
                qt_r = pool.tile([64, N], f32r, tag="qt")
                kt_r = pool.tile([64, N], f32r, tag="kt")
                nc.gpsimd.dma_start(qt_r[:], QT[p])
                nc.gpsimd.dma_start(kt_r[:], KT[p])
                v_bf = pool.tile([128, NT, 64], bf16, tag="v")
                nc.sync.dma_start(v_bf[:], Vb[p].rearrange("(t pp) d -> pp t d", pp=128))
                nct_r = pool.tile([64, M], f32r, tag="nctr")
                nrt_r = pool.tile([64, M], f32r, tag="nrtr")
                nc.gpsimd.dma_start(nct_r[:], NCT[p])
                nc.gpsimd.dma_start(nrt_r[:], NRT[p])
                nct32 = pool.tile([64, M], f32, tag="nct32")
                nrt32 = pool.tile([64, M], f32, tag="nrt32")
                nc.sync.dma_start(nct32[:], NCT[p])
                nc.sync.dma_start(nrt32[:], NRT[p])

                er = pool.tile([128, N], bf16, tag="er")
                racc = pool.tile([128, NCH], f32, tag="racc")
                for j in range(NCH):
                    ps_r = psum.tile([128, NCHUNK], f32, tag="ps_big")
                    nc.tensor.matmul(ps_r[:], nrt_r[:], kt_r[:, j*NCHUNK:(j+1)*NCHUNK],
                                     start=True, stop=True)
                    nc.scalar.activation(er[:, j*NCHUNK:(j+1)*NCHUNK], ps_r[:],
                                         AF.Exp, accum_out=racc[:, j:j+1])
                ert = pool.tile([128, NT, 128], bf16, tag="ert")
                nc.sync.dma_start_transpose(ert[:], er[:])
                ps_S = pss.tile([128, 64], f32, tag="ps_s")
                for t in range(NT):
                    nc.tensor.matmul(ps_S[:], ert[:, t, :], v_bf[:, t, :],
                                     start=(t == 0), stop=(t == NT - 1))
                rsum = pool.tile([128, 1], f32, tag="rsum")
                nc.scalar.activation(racc[:], racc[:], AF.Copy, accum_out=rsum[:])
                rrec = pool.tile([128, 1], f32, tag="rrec")
                nc.vector.reciprocal(rrec[:], rsum[:])
                s_bf = pool.tile([128, 64], bf16, tag="s_bf")
                nc.vector.tensor_scalar_mul(s_bf[:], ps_S[:], rrec[:])

                ps_m = pss.tile([128, 128], f32, tag="ps_m")
                nc.tensor.matmul(ps_m[:], nrt32[:], nct32[:], start=True, stop=True)
                e_m = pool.tile([128, 128], f32, tag="e_m")
                msum = pool.tile([128, 1], f32, tag="msum")
                nc.scalar.activation(e_m[:], ps_m[:], AF.Exp, accum_out=msum[:])
                mrec = pool.tile([128, 1], f32, tag="mrec")
                nc.vector.reciprocal(mrec[:], msum[:])
                k2_bf = pool.tile([128, 128], bf16, tag="k2")
                nc.vector.tensor_scalar_mul(k2_bf[:], e_m[:], mrec[:])

                ps_t = pss.tile([128, 128], bf16, tag="ps_m")
                nc.tensor.transpose(ps_t[:], k2_bf[:], ident_bf[:])
                k2t_bf = pool.tile([128, 128], bf16, tag="k2t")
                nc.scalar.copy(k2t_bf[:], ps_t[:])
                vm_bf = poold.tile([128, 128], bf16, tag="vm")
                nc.vector.tensor_scalar_mul(vm_bf[:], ps_t[:], gsb[:])
                for it in range(6):
                    ps_kv = pss.tile([128, 128], f32, tag="ps_m")
                    nc.tensor.matmul(ps_kv[:], k2t_bf[:], vm_bf[:], start=True, stop=True)
                    kv_bf = poold.tile([128, 128], bf16, tag="kv")
                    nc.scalar.copy(kv_bf[:], ps_kv[:])
                    t1 = poold.tile([128, 128], bf16, tag="t1")
                    nc.vector.tensor_sub(t1[:], i7[:], kv_bf[:])
                    ps_kvt = pss.tile([128, 128], bf16, tag="ps_m")
                    nc.tensor.transpose(ps_kvt[:], kv_bf[:], ident_bf[:])
                    kvt_bf = poold.tile([128, 128], bf16, tag="kvt")
                    nc.scalar.copy(kvt_bf[:], ps_kvt[:])
                    ps_t2 = pss.tile([128, 128], f32, tag="ps_m")
                    nc.tensor.matmul(ps_t2[:], kvt_bf[:], t1[:], start=True, stop=True)
                    t3 = poold.tile([128, 128], bf16, tag="t3")
                    nc.vector.tensor_sub(t3[:], i15[:], ps_t2[:])
                    ps_t4 = pss.tile([128, 128], f32, tag="ps_m")
                    nc.tensor.matmul(ps_t4[:], kvt_bf[:], t3[:], start=True, stop=True)
                    t5 = poold.tile([128, 128], bf16, tag="t5")
                    nc.vector.tensor_sub(t5[:], i13[:], ps_t4[:])
                    ps_vt = pss.tile([128, 128], bf16, tag="ps_m")
                    nc.tensor.transpose(ps_vt[:], vm_bf[:], ident_bf[:])
                    vmt_bf = poold.tile([128, 128], bf16, tag="vmt")
                    nc.scalar.copy(vmt_bf[:], ps_vt[:])
                    ps_vn = pss.tile([128, 128], f32, tag="ps_m")
                    nc.tensor.matmul(ps_vn[:], vmt_bf[:], t5[:], start=True, stop=True)
                    vm_bf = poold.tile([128, 128], bf16, tag="vm")
                    nc.vector.tensor_scalar(vm_bf[:], ps_vn[:], 0.25, scalar2=None,
                                            op0=mybir.AluOpType.mult)
                ps_vt2 = pss.tile([128, 128], bf16, tag="ps_m")
                nc.tensor.transpose(ps_vt2[:], vm_bf[:], ident_bf[:])
                vmt2 = poold.tile([128, 128], bf16, tag="vmt2")
                nc.scalar.copy(vmt2[:], ps_vt2[:])
                ps_A = pss.tile([128, 64], f32, tag="ps_a")
                nc.tensor.matmul(ps_A[:], vmt2[:], s_bf[:], start=True, stop=True)
                b_bf = pool.tile([128, 65], bf16, tag="b_bf")
                nc.vector.memset(b_bf[:, 64:65], 1.0)
                nc.vector.tensor_copy(b_bf[:, 0:64], ps_A[:])

                for j in range(NCH):
                    ps_c = psum.tile([128, NCHUNK], f32, tag="ps_big")
                    nc.tensor.matmul(ps_c[:], nct_r[:], qt_r[:, j*NCHUNK:(j+1)*NCHUNK],
                                     start=True, stop=True)
                    ec = poold.tile([128, NCHUNK], bf16, tag="ec")
                    nc.scalar.activation(ec[:], ps_c[:], AF.Exp)
                    ps_X = psum.tile([128, 4, 65], f32, tag="ps_x")
                    for t in range(4):
                        nc.tensor.matmul(ps_X[:, t, :], ec[:, t*128:(t+1)*128], b_bf[:],
                                         start=True, stop=True)
                    xrec = poold.tile([128, 4], f32, tag="xrec")
                    nc.vector.reciprocal(xrec[:], ps_X[:, :, 64])
                    xout = poold.tile([128, 4, 64], f32, tag="xout")
                    nc.vector.tensor_tensor(out=xout[:], in0=ps_X[:, :, 0:64],
                        in1=xrec.rearrange("p (t o) -> p t o", o=1).to_broadcast([128, 4, 64]),
                        op=mybir.AluOpType.mult)
                    nc.sync.dma_start(
                        XO[p, j*NCHUNK:(j+1)*NCHUNK, :].rearrange("(t pp) d -> pp t d", pp=128),
                        xout[:])
    nc.finalize()
    _cache["nc"] = nc
    return nc


def kernel(Q, K, V, mask):
    from concourse.bass_utils import run_bass_kernel_spmd

    Q = np.asarray(Q, dtype=np.float32)
    K = np.asarray(K, dtype=np.float32)
    V = np.asarray(V, dtype=np.float32)
    Qf = Q.reshape(B * H, N, D)
    Kf = K.reshape(B * H, N, D)
    Vf = V.reshape(B * H, N, D)

    nct = np.empty((B * H, D, M), np.float32)
    nrt = np.empty((B * H, D, M), np.float32)
    gmax = 0.0
    for i in range(B * H):
        for (T, out) in ((Kf, nct), (Qf, nrt)):
            s = T[i, :, 0].copy()
            s[0] = np.inf
            idx = np.argpartition(-s, M)[:M]
            out[i] = T[i, np.sort(idx), :].T
        nr = nrt[i].T.astype(np.float64)
        nc_ = nct[i].T.astype(np.float64)
        m = nr @ nc_.T
        e = np.exp(m - m.max(axis=1, keepdims=True))
        k2 = e / e.sum(axis=1, keepdims=True)
        gmax = max(gmax, float(k2.sum(axis=0).max()))

    QTf = np.ascontiguousarray(Qf.transpose(0, 2, 1))
    KTf = np.ascontiguousarray(Kf.transpose(0, 2, 1))
    Vbf = Vf.astype(ml_dtypes.bfloat16)
    gs = np.array([[1.0 / gmax]], np.float32)

    nc = _build()
    in_maps = []
    for c in range(NCORES):
        sl = slice(c * PAIRS, (c + 1) * PAIRS)
        in_maps.append({"QT": QTf[sl], "KT": KTf[sl], "Vb": Vbf[sl],
                        "NCT": nct[sl], "NRT": nrt[sl], "GS": gs})
    res = run_bass_kernel_spmd(nc, in_maps, list(range(NCORES)))
    globals()["_last_results"] = res
    X = np.concatenate([res.results[c]["XO"] for c in range(NCORES)], axis=0)
    return X.reshape(B, H, N, D)



# revision 13
# speedup vs baseline: 2.7944x; 2.7944x over previous
import numpy as np
import ml_dtypes

B, H, N, D = 4, 12, 8192, 64
M = 128
NCORES = 8
PAIRS = (B * H) // NCORES
NB = N // 128          # 64 N-blocks of 128 rows
NCH = N // 512         # 16 chunks of 512

_cache = {}


def _build():
    if "nc" in _cache:
        return _cache["nc"]
    import concourse.bacc as bacc
    import concourse.mybir as mybir
    import concourse.tile as tile

    f32 = mybir.dt.float32
    f16 = mybir.dt.float16
    bf16 = mybir.dt.bfloat16
    AF = mybir.ActivationFunctionType

    nc = bacc.Bacc("TRN2", target_bir_lowering=False, debug=False)
    QTP = nc.declare_dram_parameter("QTP", [PAIRS, 64, N], f16, isOutput=False)
    KTP = nc.declare_dram_parameter("KTP", [PAIRS, 64, N], f16, isOutput=False)
    VB = nc.declare_dram_parameter("VB", [PAIRS, 128, NB, 65], bf16, isOutput=False)
    NRT = nc.declare_dram_parameter("NRT", [64, PAIRS, 128], f16, isOutput=False)
    NCT = nc.declare_dram_parameter("NCT", [64, PAIRS, 128], f16, isOutput=False)
    GS = nc.declare_dram_parameter("GS", [1, 1], f32, isOutput=False)
    XO = nc.declare_dram_parameter("XO", [PAIRS, 128, NB, 64], bf16, isOutput=True)

    with tile.TileContext(nc) as tc:
        with (tc.tile_pool(name="pc", bufs=1) as pc,
              tc.tile_pool(name="pio", bufs=2) as pio,
              tc.tile_pool(name="pw", bufs=2) as pw,
              tc.tile_pool(name="pe3", bufs=3) as pe3,
              tc.tile_pool(name="psbig", bufs=2, space="PSUM") as psbig,
              tc.tile_pool(name="pss", bufs=1, space="PSUM") as pss):

            # ---- PSUM static regions ----
            xns = pss.tile([128, 2048], f32, tag="xns")
            SACC = xns[:, 0:512]          # S accumulate [:, 0:65]; X slots 0-3
            KVR = xns[:, 512:1280]        # NS kv/ps3/ps4; m-phase; A; X slots 4-9
            KVTR = xns[:, 1280:2048]      # NS kvt/ps5/ps6; X slots 10-15
            kvtr_bc = xns[:].bitcast(bf16)[:, 2560:4096]  # bf16 view of KVTR

            # ---- constants ----
            # tiny exp to pull the ACT table load off the critical path
            warm = pc.tile([1, 1], f32, tag="warm")
            nc.vector.memset(warm[:], 0.0)
            nc.scalar.activation(warm[:], warm[:], AF.Exp)

            ident = pc.tile([128, 128], bf16, tag="ident")
            nc.gpsimd.memset(ident[:], 0.0)
            nc.gpsimd.affine_select(out=ident[:], in_=ident[:],
                compare_op=mybir.AluOpType.not_equal, fill=1.0, base=0,
                pattern=[[-1, 128]], channel_multiplier=1)
            diag6 = {}
            for val in (7.0, 15.0, 13.0):
                t6 = pc.tile([128, PAIRS * 128], bf16, tag=f"i{int(val)}_6",
                             name=f"i{int(val)}_6")
                nc.gpsimd.memset(t6[:], 0.0)
                for q in range(PAIRS):
                    sl = t6[:, q * 128:(q + 1) * 128]
                    nc.gpsimd.affine_select(out=sl, in_=sl,
                        compare_op=mybir.AluOpType.not_equal, fill=val, base=0,
                        pattern=[[-1, 128]], channel_multiplier=1)
                diag6[val] = t6
            i7_6, i15_6, i13_6 = diag6[7.0], diag6[15.0], diag6[13.0]

            nrt_all = pc.tile([64, PAIRS, 128], f16, tag="nrt")
            nct_all = pc.tile([64, PAIRS, 128], f16, tag="nct")
            nc.sync.dma_start(nrt_all[:], NRT[:])
            nc.sync.dma_start(nct_all[:], NCT[:])
            ones_row = pc.tile([1, 128], f32, tag="ones_row")
            nc.vector.memset(ones_row[:], 1.0)
            gs_sb = pc.tile([1, 1], f32, tag="gs_sb")
            nc.sync.dma_start(gs_sb[:], GS[:])

            ps_gs = xns[:, 448:449]
            nc.tensor.matmul(ps_gs, ones_row[:], gs_sb[:], start=True, stop=True)
            gsb = pc.tile([128, 1], f32, tag="gsb")
            nc.vector.tensor_copy(gsb[:], ps_gs)

            ball = pc.tile([128, PAIRS, 65], bf16, tag="ball")
            s_bf6 = pc.tile([128, PAIRS, 64], bf16, tag="s_bf6")

            # ---- m / k2 prep for ALL pairs (landmarks only) ----
            e_m6 = pc.tile([128, PAIRS * 128], f32, tag="e_m6")
            msum6 = pc.tile([128, PAIRS], f32, tag="msum6")
            for q in range(PAIRS):
                nc.tensor.matmul(KVR[:, q * 128:(q + 1) * 128],
                                 nrt_all[:, q, :], nct_all[:, q, :],
                                 start=True, stop=True)
            for q in range(PAIRS):
                nc.scalar.activation(e_m6[:, q * 128:(q + 1) * 128],
                                     KVR[:, q * 128:(q + 1) * 128],
                                     AF.Exp, accum_out=msum6[:, q:q + 1])
            mrec6 = pc.tile([128, PAIRS, 1], f32, tag="mrec6")
            nc.vector.reciprocal(mrec6[:], msum6[:].rearrange(
                "p (q o) -> p q o", o=1))
            k2_6 = pc.tile([128, PAIRS * 128], bf16, tag="k2_6")
            nc.vector.tensor_tensor(
                out=k2_6[:].rearrange("p (q e) -> p q e", e=128),
                in0=e_m6[:].rearrange("p (q e) -> p q e", e=128),
                in1=mrec6[:].to_broadcast([128, PAIRS, 128]),
                op=mybir.AluOpType.mult)
            for q in range(PAIRS):
                nc.tensor.transpose(kvtr_bc[:, q * 256:q * 256 + 128],
                                    k2_6[:, q * 128:(q + 1) * 128], ident[:])
            kvtr_slots = kvtr_bc.rearrange("p (q e) -> p q e", e=256)[:, :, 0:128]
            k2t6 = pc.tile([128, PAIRS * 128], bf16, tag="k2t6")
            nc.vector.tensor_copy(
                k2t6[:].rearrange("p (q e) -> p q e", e=128), kvtr_slots)
            vm6 = pw.tile([128, PAIRS * 128], bf16, tag="vm6")
            nc.vector.tensor_scalar_mul(
                vm6[:].rearrange("p (q e) -> p q e", e=128), kvtr_slots, gsb[:])
            vmt6 = pw.tile([128, PAIRS * 128], bf16, tag="vmt6")
            nc.vector.tensor_scalar_mul(vmt6[:], k2_6[:], gsb[:])

            # ---- Newton-Schulz stages (one iteration = 5 stages, spread over
            # er-phase steps so the PE never queues behind a pending DVE op) --
            ns_state = {"vm6": vm6, "vmt6": vmt6, "u1": None, "u3": None,
                        "u5": None, "kvt6": None}

            def ns_stage(j, stage):
                vm6, vmt6 = ns_state["vm6"], ns_state["vmt6"]
                if stage == 0:
                    for q in range(PAIRS):
                        nc.tensor.matmul(KVR[:, q * 128:(q + 1) * 128],
                                         k2t6[:, q * 128:(q + 1) * 128],
                                         vm6[:, q * 128:(q + 1) * 128],
                                         start=True, stop=True)
                    for q in range(PAIRS):
                        nc.tensor.matmul(KVTR[:, q * 128:(q + 1) * 128],
                                         vm6[:, q * 128:(q + 1) * 128],
                                         k2t6[:, q * 128:(q + 1) * 128],
                                         start=True, stop=True)
                    kvt6 = pw.tile([128, PAIRS * 128], bf16, tag="kvt6",
                                   name="kvt6")
                    nc.vector.tensor_copy(kvt6[:], KVTR)
                    u1 = pw.tile([128, PAIRS * 128], bf16, tag="u1", name="u1")
                    nc.vector.tensor_sub(u1[:], i7_6[:], KVR)
                    ns_state["kvt6"], ns_state["u1"] = kvt6, u1
                elif stage == 1:
                    kvt6, u1 = ns_state["kvt6"], ns_state["u1"]
                    for q in range(PAIRS):
                        nc.tensor.matmul(KVR[:, q * 128:(q + 1) * 128],
                                         kvt6[:, q * 128:(q + 1) * 128],
                                         u1[:, q * 128:(q + 1) * 128],
                                         start=True, stop=True)
                    u3 = pw.tile([128, PAIRS * 128], bf16, tag="u3", name="u3")
                    nc.vector.tensor_sub(u3[:], i15_6[:], KVR)
                    ns_state["u3"] = u3
                elif stage == 2:
                    kvt6, u3 = ns_state["kvt6"], ns_state["u3"]
                    for q in range(PAIRS):
                        nc.tensor.matmul(KVR[:, q * 128:(q + 1) * 128],
                                         kvt6[:, q * 128:(q + 1) * 128],
                                         u3[:, q * 128:(q + 1) * 128],
                                         start=True, stop=True)
                    u5 = pw.tile([128, PAIRS * 128], bf16, tag="u5", name="u5")
                    nc.vector.tensor_sub(u5[:], i13_6[:], KVR)
                    ns_state["u5"] = u5
                elif stage == 3:
                    u5 = ns_state["u5"]
                    if j < 5:
                        for q in range(PAIRS):
                            nc.tensor.matmul(KVTR[:, q * 128:(q + 1) * 128],
                                             vmt6[:, q * 128:(q + 1) * 128],
                                             u5[:, q * 128:(q + 1) * 128],
                                             start=True, stop=True)
                        vm6n = pw.tile([128, PAIRS * 128], bf16, tag="vm6",
                                       name="vm6")
                        nc.vector.tensor_scalar(vm6n[:], KVTR, 0.25,
                                                scalar2=None,
                                                op0=mybir.AluOpType.mult)
                        ns_state["vm6"] = vm6n
                elif stage == 4:
                    u5 = ns_state["u5"]
                    for q in range(PAIRS):
                        nc.tensor.matmul(KVTR[:, q * 128:(q + 1) * 128],
                                         u5[:, q * 128:(q + 1) * 128],
                                         vmt6[:, q * 128:(q + 1) * 128],
                                         start=True, stop=True)
                    vmt6n = pw.tile([128, PAIRS * 128], bf16, tag="vmt6",
                                    name="vmt6")
                    nc.vector.tensor_scalar(vmt6n[:], KVTR, 0.25, scalar2=None,
                                            op0=mybir.AluOpType.mult)
                    ns_state["vmt6"] = vmt6n

            # ---------------- sweep 1: er logits -> exp -> S ----------------
            kt_t = [None] * PAIRS
            vb_t = [None] * PAIRS
            qt_t = [None] * PAIRS

            def load_kv(p):
                kt_t[p] = pio.tile([64, N], f16, tag="kt", name="kt")
                nc.sync.dma_start(kt_t[p][:], KTP[p])
                vb_t[p] = pio.tile([128, NB, 65], bf16, tag="vb", name="vb")
                nc.sync.dma_start(vb_t[p][:], VB[p])

            def load_q(p):
                qt_t[p] = pio.tile([64, N], f16, tag="qt", name="qt")
                nc.sync.dma_start(qt_t[p][:], QTP[p])

            load_kv(0)
            load_kv(1)

            for p in range(PAIRS):
                if p + 2 < PAIRS:
                    load_kv(p + 2)
                nr = nrt_all[:, p, :]
                kt = kt_t[p]
                vb = vb_t[p]

                erc_t = [None] * 8
                for k in range(9):
                    if k < 8:
                        ps_er = psbig.tile([128, 1024], f32, tag="big",
                                           name="ps_er")
                        for j in range(8):
                            blk = 8 * k + j
                            nc.tensor.matmul(
                                ps_er[:, j * 128:(j + 1) * 128],
                                kt[:, blk * 128:(blk + 1) * 128],
                                nr, start=True, stop=True)
                        erc_t[k] = pe3.tile([128, 1024], bf16, tag="erc",
                                            name="erc")
                        nc.scalar.activation(erc_t[k][:], ps_er[:], AF.Exp)
                    if k >= 1:
                        kc = k - 1
                        erc = erc_t[kc]
                        for s in range(8):
                            t = 8 * kc + s
                            nc.tensor.matmul(
                                SACC[:, 0:65], erc[:, s * 128:(s + 1) * 128],
                                vb[:, t, :],
                                start=(kc == 0 and s == 0),
                                stop=(kc == 7 and s == 7),
                                skip_group_check=True)
                        erc_t[kc] = None
                    if 2 <= k <= 6:
                        ns_stage(p, k - 2)

                srec = pw.tile([128, 1], f32, tag="srec")
                nc.vector.reciprocal(srec[:], SACC[:, 64:65])
                nc.vector.tensor_scalar_mul(s_bf6[:, p, :], SACC[:, 0:64],
                                            srec[:])
                kt_t[p] = None
                vb_t[p] = None
                if p == 3:
                    load_q(0)
                if p == 4:
                    load_q(1)

            # ---- A = Vm_final @ S_norm; b = [A | 1] ----
            vmt6 = ns_state["vmt6"]
            for q in range(PAIRS):
                nc.tensor.matmul(KVR[:, q * 128:q * 128 + 64],
                                 vmt6[:, q * 128:(q + 1) * 128],
                                 s_bf6[:, q, :], start=True, stop=True)
            nc.vector.tensor_copy(
                ball[:, :, 0:64],
                KVR.rearrange("p (q e) -> p q e", e=128)[:, :, 0:64])
            nc.vector.memset(ball[:, :, 64:65], 1.0)

            # ---------------- sweep 2: ec logits -> exp -> X ----------------
            xv = xns[:].rearrange("p (s e) -> p s e", e=128)
            for p in range(PAIRS):
                if p + 2 < PAIRS:
                    load_q(p + 2)
                qt = qt_t[p]
                ncs = nct_all[:, p, :]
                b_p = ball[:, p, :]
                xsb = pio.tile([128, NB, 64], bf16, tag="xsb", name="xsb")
                ec_t = [None] * 8
                for i in range(9):
                    if i < 8:
                        ps_ec = psbig.tile([128, 1024], f32, tag="big",
                                           name="ps_ec")
                        nc.tensor.matmul(ps_ec[:, 0:512], ncs,
                                         qt[:, (2 * i) * 512:(2 * i + 1) * 512],
                                         start=True, stop=True)
                        nc.tensor.matmul(ps_ec[:, 512:1024], ncs,
                                         qt[:, (2 * i + 1) * 512:(2 * i + 2) * 512],
                                         start=True, stop=True)
                        ec_t[i] = pe3.tile([128, 1024], bf16, tag="ec",
                                           name="ec")
                        nc.scalar.activation(ec_t[i][:], ps_ec[:], AF.Exp)
                    if i >= 1:
                        ic = i - 1
                        ec_sb = ec_t[ic]
                        for c in range(8):
                            sl = (ic % 2) * 8 + c
                            nc.tensor.matmul(
                                xns[:, sl * 128:sl * 128 + 65],
                                ec_sb[:, c * 128:(c + 1) * 128], b_p,
                                start=True, stop=True)
                        ec_t[ic] = None
                        if ic % 2 == 1:
                            m2 = ic // 2
                            rec = pw.tile([128, 16, 1], f32, tag="rec")
                            nc.vector.reciprocal(rec[:], xv[:, :, 64:65])
                            nc.vector.tensor_tensor(
                                out=xsb[:, 16 * m2:16 * (m2 + 1), :],
                                in0=xv[:, :, 0:64],
                                in1=rec[:].to_broadcast([128, 16, 64]),
                                op=mybir.AluOpType.mult)
                nc.sync.dma_start(XO[p], xsb[:])
                qt_t[p] = None
    nc.finalize()
    _cache["nc"] = nc
    return nc


def kernel(Q, K, V, mask):
    from concourse.bass_utils import run_bass_kernel_spmd

    Q = np.asarray(Q, dtype=np.float32)
    K = np.asarray(K, dtype=np.float32)
    V = np.asarray(V, dtype=np.float32)
    Qf = Q.reshape(B * H, N, D)
    Kf = K.reshape(B * H, N, D)
    Vf = V.reshape(B * H, N, D)

    nct = np.empty((B * H, D, M), np.float32)
    nrt = np.empty((B * H, D, M), np.float32)
    gmax = 0.0
    for i in range(B * H):
        for (T, out) in ((Kf, nct), (Qf, nrt)):
            s = T[i, :, 0].copy()
            s[0] = np.inf
            idx = np.argpartition(-s, M)[:M]
            out[i] = T[i, np.sort(idx), :].T
        nr = nrt[i].T.astype(np.float64)
        nc_ = nct[i].T.astype(np.float64)
        m = nr @ nc_.T
        e = np.exp(m - m.max(axis=1, keepdims=True))
        k2 = e / e.sum(axis=1, keepdims=True)
        gmax = max(gmax, float(k2.sum(axis=0).max()))

    QTp = np.ascontiguousarray(Qf.transpose(0, 2, 1)).astype(np.float16)
    KTp = np.ascontiguousarray(Kf.transpose(0, 2, 1)).astype(np.float16)

    Vext = np.concatenate([Vf, np.ones((B * H, N, 1), np.float32)], axis=2)
    VBp = np.ascontiguousarray(
        Vext.reshape(B * H, NB, 128, 65).transpose(0, 2, 1, 3)
    ).astype(ml_dtypes.bfloat16)

    NRTp = nrt.astype(np.float16)   # [48, 64, 128]
    NCTp = nct.astype(np.float16)
    gs = np.array([[1.0 / gmax]], np.float32)

    nc = _build()
    in_maps = []
    for c in range(NCORES):
        sl = slice(c * PAIRS, (c + 1) * PAIRS)
        in_maps.append({
            "QTP": QTp[sl], "KTP": KTp[sl], "VB": VBp[sl],
            "NRT": np.ascontiguousarray(NRTp[sl].transpose(1, 0, 2)),
            "NCT": np.ascontiguousarray(NCTp[sl].transpose(1, 0, 2)),
            "GS": gs,
        })
    res = run_bass_kernel_spmd(nc, in_maps, list(range(NCORES)))
    globals()["_last_results"] = res
    XOa = np.stack([res.results[c]["XO"] for c in range(NCORES)], axis=0)
    XOa = XOa.reshape(B * H, 128, NB, 64).astype(np.float32)
    X = XOa.transpose(0, 2, 1, 3).reshape(B * H, N, D)
    return np.ascontiguousarray(X).reshape(B, H, N, D)


# revision 21
# speedup vs baseline: 3.0044x; 1.0751x over previous
import numpy as np
import ml_dtypes

B, H, N, D = 4, 12, 8192, 64
M = 128
NCORES = 8
PAIRS = (B * H) // NCORES
NB = N // 128          # 64 N-blocks of 128 rows
NCH = N // 512         # 16 chunks of 512

_cache = {}


def _build():
    if "nc" in _cache:
        return _cache["nc"]
    import concourse.bacc as bacc
    import concourse.mybir as mybir
    import concourse.tile as tile

    f32 = mybir.dt.float32
    f16 = mybir.dt.float16
    bf16 = mybir.dt.bfloat16
    AF = mybir.ActivationFunctionType

    nc = bacc.Bacc("TRN2", target_bir_lowering=False, debug=False)
    QTP = nc.declare_dram_parameter("QTP", [PAIRS, 64, N], f16, isOutput=False)
    KTP = nc.declare_dram_parameter("KTP", [PAIRS, 64, N], f16, isOutput=False)
    VB = nc.declare_dram_parameter("VB", [PAIRS, 128, NB, 65], bf16, isOutput=False)
    NRT = nc.declare_dram_parameter("NRT", [64, PAIRS, 128], f16, isOutput=False)
    NCT = nc.declare_dram_parameter("NCT", [64, PAIRS, 128], f16, isOutput=False)
    GS = nc.declare_dram_parameter("GS", [1, 1], f32, isOutput=False)
    XO = nc.declare_dram_parameter("XO", [PAIRS, 128, NB, 64], bf16, isOutput=True)

    with tile.TileContext(nc) as tc:
        with (tc.tile_pool(name="pc", bufs=1) as pc,
              tc.tile_pool(name="pio", bufs=2) as pio,
              tc.tile_pool(name="pw", bufs=2) as pw,
              tc.tile_pool(name="pe3", bufs=3) as pe3,
              tc.tile_pool(name="psbig", bufs=2, space="PSUM") as psbig,
              tc.tile_pool(name="pss", bufs=1, space="PSUM") as pss):

            # ---- PSUM static regions ----
            xns = pss.tile([128, 2048], f32, tag="xns")
            SACC = xns[:, 0:512]          # S accumulate [:, 0:65]; X slots 0-3
            KVR = xns[:, 512:1280]        # NS kv/ps3/ps4; m-phase; A; X slots 4-9
            KVTR = xns[:, 1280:2048]      # NS kvt/ps5/ps6; X slots 10-15
            kvtr_bc = xns[:].bitcast(bf16)[:, 2560:4096]  # bf16 view of KVTR

            # ---- constants ----
            # tiny exp to pull the ACT table load off the critical path
            warm = pc.tile([1, 1], f32, tag="warm")
            nc.vector.memset(warm[:], 0.0)
            nc.scalar.activation(warm[:], warm[:], AF.Exp)

            ident = pc.tile([128, 128], bf16, tag="ident")
            nc.gpsimd.memset(ident[:], 0.0)
            nc.gpsimd.affine_select(out=ident[:], in_=ident[:],
                compare_op=mybir.AluOpType.not_equal, fill=1.0, base=0,
                pattern=[[-1, 128]], channel_multiplier=1)
            diag6 = {}
            for val in (7.0, 15.0, 13.0):
                t6 = pc.tile([128, PAIRS * 128], bf16, tag=f"i{int(val)}_6",
                             name=f"i{int(val)}_6")
                nc.gpsimd.memset(t6[:], 0.0)
                for q in range(PAIRS):
                    sl = t6[:, q * 128:(q + 1) * 128]
                    nc.gpsimd.affine_select(out=sl, in_=sl,
                        compare_op=mybir.AluOpType.not_equal, fill=val, base=0,
                        pattern=[[-1, 128]], channel_multiplier=1)
                diag6[val] = t6
            i7_6, i15_6, i13_6 = diag6[7.0], diag6[15.0], diag6[13.0]

            nrt_all = pc.tile([64, PAIRS, 128], f16, tag="nrt")
            nct_all = pc.tile([64, PAIRS, 128], f16, tag="nct")
            nc.sync.dma_start(nrt_all[:], NRT[:])
            nc.sync.dma_start(nct_all[:], NCT[:])
            ones_row = pc.tile([1, 128], f32, tag="ones_row")
            nc.vector.memset(ones_row[:], 1.0)
            gs_sb = pc.tile([1, 1], f32, tag="gs_sb")
            nc.sync.dma_start(gs_sb[:], GS[:])

            ps_gs = xns[:, 448:449]
            nc.tensor.matmul(ps_gs, ones_row[:], gs_sb[:], start=True, stop=True)
            gsb = pc.tile([128, 1], f32, tag="gsb")
            nc.vector.tensor_copy(gsb[:], ps_gs)

            ball = pc.tile([128, PAIRS, 65], bf16, tag="ball")
            s_bf6 = pc.tile([128, PAIRS, 64], bf16, tag="s_bf6")

            # ---- m / k2 prep for ALL pairs: matmuls up front, the rest is
            # injected into the stream at slot 1 so the first er logits are
            # not queued behind the transposes.
            e_m6 = pc.tile([128, PAIRS * 128], f32, tag="e_m6")
            msum6 = pc.tile([128, PAIRS], f32, tag="msum6")
            k2t6 = pc.tile([128, PAIRS * 128], bf16, tag="k2t6")
            k2_6 = pc.tile([128, PAIRS * 128], bf16, tag="k2_6")
            for q in range(PAIRS):
                nc.tensor.matmul(KVR[:, q * 128:(q + 1) * 128],
                                 nrt_all[:, q, :], nct_all[:, q, :],
                                 start=True, stop=True)
            ns_state = {}

            # ACT/DVE part of the m-phase runs before the stream (fills the
            # first kt DMA wait); the PE transposes are injected at slot 1.
            for q in range(PAIRS):
                nc.scalar.activation(e_m6[:, q * 128:(q + 1) * 128],
                                     KVR[:, q * 128:(q + 1) * 128],
                                     AF.Exp, accum_out=msum6[:, q:q + 1])
            mrec6 = pc.tile([128, PAIRS, 1], f32, tag="mrec6")
            nc.vector.reciprocal(mrec6[:], msum6[:].rearrange(
                "p (q o) -> p q o", o=1))
            nc.vector.tensor_tensor(
                out=k2_6[:].rearrange("p (q e) -> p q e", e=128),
                in0=e_m6[:].rearrange("p (q e) -> p q e", e=128),
                in1=mrec6[:].to_broadcast([128, PAIRS, 128]),
                op=mybir.AluOpType.mult)

            def m_phase_rest():
                for q in range(PAIRS):
                    nc.tensor.transpose(kvtr_bc[:, q * 256:q * 256 + 128],
                                        k2_6[:, q * 128:(q + 1) * 128],
                                        ident[:])
                kvtr_slots = kvtr_bc.rearrange(
                    "p (q e) -> p q e", e=256)[:, :, 0:128]
                nc.vector.tensor_copy(
                    k2t6[:].rearrange("p (q e) -> p q e", e=128), kvtr_slots)
                vm6 = pw.tile([128, PAIRS * 128], bf16, tag="vm6")
                nc.vector.tensor_scalar_mul(
                    vm6[:].rearrange("p (q e) -> p q e", e=128), kvtr_slots,
                    gsb[:])
                vmt6 = pw.tile([128, PAIRS * 128], bf16, tag="vmt6")
                nc.vector.tensor_scalar_mul(vmt6[:], k2_6[:], gsb[:])
                ns_state.update({"vm6": vm6, "vmt6": vmt6})

            # ---- Newton-Schulz stages (one iteration = 4 stages, paced one
            # stage per two er-steps so the PE never queues behind a pending
            # DVE op; each stage's PE matmuls depend only on DVE work from
            # >=2 er-steps earlier) ----

            def ns_stage(j, stage):
                vm6, vmt6 = ns_state["vm6"], ns_state["vmt6"]
                if stage == 0:
                    for q in range(PAIRS):
                        nc.tensor.matmul(KVR[:, q * 128:(q + 1) * 128],
                                         k2t6[:, q * 128:(q + 1) * 128],
                                         vm6[:, q * 128:(q + 1) * 128],
                                         start=True, stop=True)
                    for q in range(PAIRS):
                        nc.tensor.matmul(KVTR[:, q * 128:(q + 1) * 128],
                                         vm6[:, q * 128:(q + 1) * 128],
                                         k2t6[:, q * 128:(q + 1) * 128],
                                         start=True, stop=True)
                    kvt6 = pw.tile([128, PAIRS * 128], bf16, tag="kvt6",
                                   name="kvt6")
                    nc.vector.tensor_copy(kvt6[:], KVTR)
                    u1 = pw.tile([128, PAIRS * 128], bf16, tag="u1", name="u1")
                    nc.vector.tensor_sub(u1[:], i7_6[:], KVR)
                    ns_state["kvt6"], ns_state["u1"] = kvt6, u1
                elif stage == 1:
                    kvt6, u1 = ns_state["kvt6"], ns_state["u1"]
                    for q in range(PAIRS):
                        nc.tensor.matmul(KVR[:, q * 128:(q + 1) * 128],
                                         kvt6[:, q * 128:(q + 1) * 128],
                                         u1[:, q * 128:(q + 1) * 128],
                                         start=True, stop=True)
                    u3 = pw.tile([128, PAIRS * 128], bf16, tag="u3", name="u3")
                    nc.vector.tensor_sub(u3[:], i15_6[:], KVR)
                    ns_state["u3"] = u3
                elif stage == 2:
                    kvt6, u3 = ns_state["kvt6"], ns_state["u3"]
                    for q in range(PAIRS):
                        nc.tensor.matmul(KVR[:, q * 128:(q + 1) * 128],
                                         kvt6[:, q * 128:(q + 1) * 128],
                                         u3[:, q * 128:(q + 1) * 128],
                                         start=True, stop=True)
                    u5 = pw.tile([128, PAIRS * 128], bf16, tag="u5", name="u5")
                    nc.vector.tensor_sub(u5[:], i13_6[:], KVR)
                    ns_state["u5"] = u5
                elif stage == 3:
                    # ps5 -> KVTR, ps6 -> KVR: the two scale ops read
                    # different regions, so neither matmul set waits on the
                    # other's DVE op.
                    u5 = ns_state["u5"]
                    if j < 5:
                        for q in range(PAIRS):
                            nc.tensor.matmul(KVTR[:, q * 128:(q + 1) * 128],
                                             vmt6[:, q * 128:(q + 1) * 128],
                                             u5[:, q * 128:(q + 1) * 128],
                                             start=True, stop=True)
                    for q in range(PAIRS):
                        nc.tensor.matmul(KVR[:, q * 128:(q + 1) * 128],
                                         u5[:, q * 128:(q + 1) * 128],
                                         vmt6[:, q * 128:(q + 1) * 128],
                                         start=True, stop=True)
                    if j < 5:
                        vm6n = pw.tile([128, PAIRS * 128], bf16, tag="vm6",
                                       name="vm6")
                        nc.vector.tensor_scalar(vm6n[:], KVTR, 0.25,
                                                scalar2=None,
                                                op0=mybir.AluOpType.mult)
                        ns_state["vm6"] = vm6n
                    vmt6n = pw.tile([128, PAIRS * 128], bf16, tag="vmt6",
                                    name="vmt6")
                    nc.vector.tensor_scalar(vmt6n[:], KVR, 0.25, scalar2=None,
                                            op0=mybir.AluOpType.mult)
                    ns_state["vmt6"] = vmt6n

            # ---------------- sweep 1: er logits -> exp -> S ----------------
            kt_t = [None] * PAIRS
            vb_t = [None] * PAIRS
            qt_t = [None] * PAIRS

            def load_kv(p):
                kt_t[p] = pio.tile([64, N], f16, tag="kt", name="kt")
                nc.sync.dma_start(kt_t[p][:, 0:N // 2], KTP[p][:, 0:N // 2])
                vb_t[p] = pio.tile([128, NB, 65], bf16, tag="vb", name="vb")
                nc.sync.dma_start(vb_t[p][:, 0:NB // 2, :],
                                  VB[p][:, 0:NB // 2, :])
                nc.sync.dma_start(kt_t[p][:, N // 2:N], KTP[p][:, N // 2:N])
                nc.sync.dma_start(vb_t[p][:, NB // 2:NB, :],
                                  VB[p][:, NB // 2:NB, :])

            def load_q(p):
                qt_t[p] = pio.tile([64, N], f16, tag="qt", name="qt", bufs=3)
                nc.sync.dma_start(qt_t[p][:], QTP[p])

            load_kv(0)
            load_kv(1)

            # Global batch stream: producer (logits+exp) leads the consumer
            # (S matmuls) by 2 batches, so the PE always has the next pair's
            # logit matmuls queued ahead of dependency-stalled consumers.
            NBAT = PAIRS * 8
            erc_all = [None] * NBAT
            ns_emitted = 0
            for g in range(NBAT + 2):
                if g < NBAT:
                    p, k = divmod(g, 8)
                    if k == 0 and p + 2 < PAIRS:
                        load_kv(p + 2)
                    if k == 0 and p >= 3:
                        load_q(p - 3)
                    nr = nrt_all[:, p, :]
                    kt = kt_t[p]
                    ps_er = psbig.tile([128, 1024], f32, tag="big",
                                       name="ps_er")
                    for j in range(8):
                        blk = 8 * k + j
                        nc.tensor.matmul(
                            ps_er[:, j * 128:(j + 1) * 128],
                            kt[:, blk * 128:(blk + 1) * 128],
                            nr, start=True, stop=True)
                    erc_all[g] = pe3.tile([128, 1024], bf16, tag="erc",
                                          name="erc", bufs=4)
                    nc.scalar.activation(erc_all[g][:], ps_er[:], AF.Exp)
                if g >= 2:
                    gc = g - 2
                    pc_, kc = divmod(gc, 8)
                    erc = erc_all[gc]
                    vb = vb_t[pc_]
                    soff = 128 * (pc_ % 2)
                    for s in range(8):
                        t = 8 * kc + s
                        nc.tensor.matmul(
                            SACC[:, soff:soff + 65],
                            erc[:, s * 128:(s + 1) * 128],
                            vb[:, t, :],
                            start=(kc == 0 and s == 0),
                            stop=(kc == 7 and s == 7),
                            skip_group_check=True)
                    erc_all[gc] = None
                    if kc == 7:
                        srec = pw.tile([128, 1], f32, tag="srec")
                        nc.vector.reciprocal(
                            srec[:], SACC[:, soff + 64:soff + 65])
                        nc.vector.tensor_scalar_mul(
                            s_bf6[:, pc_, :], SACC[:, soff:soff + 64],
                            srec[:])
                        kt_t[pc_] = None
                        vb_t[pc_] = None
                if g == 1:
                    m_phase_rest()
                if g >= 3 and (g - 3) % 2 == 0 and ns_emitted < 24:
                    ns_stage(ns_emitted // 4, ns_emitted % 4)
                    ns_emitted += 1

            # ---- A = Vm_final @ S_norm; b = [A | 1] ----
            vmt6 = ns_state["vmt6"]
            for q in range(PAIRS):
                nc.tensor.matmul(KVR[:, q * 128:q * 128 + 64],
                                 vmt6[:, q * 128:(q + 1) * 128],
                                 s_bf6[:, q, :], start=True, stop=True)
            nc.vector.tensor_copy(
                ball[:, :, 0:64],
                KVR.rearrange("p (q e) -> p q e", e=128)[:, :, 0:64])
            nc.vector.memset(ball[:, :, 64:65], 1.0)

            # ---------------- sweep 2: ec logits -> exp -> X ----------------
            # Same global-stream structure, consumer lag 2; output DMA is
            # chunked per division group so the store overlaps compute.
            xv = xns[:].rearrange("p (s e) -> p s e", e=128)
            xsb_t = [None] * PAIRS
            ec_all = [None] * NBAT
            for g in range(NBAT + 2):
                if g < NBAT:
                    p, i = divmod(g, 8)
                    if i == 0 and p + 3 < PAIRS:
                        load_q(p + 3)
                    if i == 0:
                        xsb_t[p] = pio.tile([128, NB, 64], bf16, tag="xsb",
                                            name="xsb")
                    qt = qt_t[p]
                    ncs = nct_all[:, p, :]
                    ps_ec = psbig.tile([128, 1024], f32, tag="big",
                                       name="ps_ec")
                    nc.tensor.matmul(ps_ec[:, 0:512], ncs,
                                     qt[:, (2 * i) * 512:(2 * i + 1) * 512],
                                     start=True, stop=True)
                    nc.tensor.matmul(ps_ec[:, 512:1024], ncs,
                                     qt[:, (2 * i + 1) * 512:(2 * i + 2) * 512],
                                     start=True, stop=True)
                    ec_all[g] = pe3.tile([128, 1024], bf16, tag="ec",
                                         name="ec", bufs=4)
                    nc.scalar.activation(ec_all[g][:], ps_ec[:], AF.Exp)
                if g >= 2:
                    gc = g - 2
                    pc_, ic = divmod(gc, 8)
                    ec_sb = ec_all[gc]
                    b_p = ball[:, pc_, :]
                    xsb = xsb_t[pc_]
                    for c in range(8):
                        sl = (ic % 2) * 8 + c
                        nc.tensor.matmul(
                            xns[:, sl * 128:sl * 128 + 65],
                            ec_sb[:, c * 128:(c + 1) * 128], b_p,
                            start=True, stop=True)
                    ec_all[gc] = None
                    if ic % 2 == 1:
                        m2 = ic // 2
                        rec = pw.tile([128, 16, 1], f32, tag="rec")
                        nc.vector.reciprocal(rec[:], xv[:, :, 64:65])
                        nc.vector.tensor_tensor(
                            out=xsb[:, 16 * m2:16 * (m2 + 1), :],
                            in0=xv[:, :, 0:64],
                            in1=rec[:].to_broadcast([128, 16, 64]),
                            op=mybir.AluOpType.mult)
                        nc.sync.dma_start(
                            XO[pc_][:, 16 * m2:16 * (m2 + 1), :],
                            xsb[:, 16 * m2:16 * (m2 + 1), :])
                    if ic == 7:
                        qt_t[pc_] = None
    nc.finalize()
    _cache["nc"] = nc
    return nc


def kernel(Q, K, V, mask):
    from concourse.bass_utils import run_bass_kernel_spmd

    Q = np.asarray(Q, dtype=np.float32)
    K = np.asarray(K, dtype=np.float32)
    V = np.asarray(V, dtype=np.float32)
    Qf = Q.reshape(B * H, N, D)
    Kf = K.reshape(B * H, N, D)
    Vf = V.reshape(B * H, N, D)

    nct = np.empty((B * H, D, M), np.float32)
    nrt = np.empty((B * H, D, M), np.float32)
    gmax = 0.0
    for i in range(B * H):
        for (T, out) in ((Kf, nct), (Qf, nrt)):
            s = T[i, :, 0].copy()
            s[0] = np.inf
            idx = np.argpartition(-s, M)[:M]
            out[i] = T[i, np.sort(idx), :].T
        nr = nrt[i].T.astype(np.float64)
        nc_ = nct[i].T.astype(np.float64)
        m = nr @ nc_.T
        e = np.exp(m - m.max(axis=1, keepdims=True))
        k2 = e / e.sum(axis=1, keepdims=True)
        gmax = max(gmax, float(k2.sum(axis=0).max()))

    QTp = np.ascontiguousarray(Qf.transpose(0, 2, 1)).astype(np.float16)
    KTp = np.ascontiguousarray(Kf.transpose(0, 2, 1)).astype(np.float16)

    Vext = np.concatenate([Vf, np.ones((B * H, N, 1), np.float32)], axis=2)
    VBp = np.ascontiguousarray(
        Vext.reshape(B * H, NB, 128, 65).transpose(0, 2, 1, 3)
    ).astype(ml_dtypes.bfloat16)

    NRTp = nrt.astype(np.float16)   # [48, 64, 128]
    NCTp = nct.astype(np.float16)
    gs = np.array([[1.0 / gmax]], np.float32)

    nc = _build()
    in_maps = []
    for c in range(NCORES):
        sl = slice(c * PAIRS, (c + 1) * PAIRS)
        in_maps.append({
            "QTP": QTp[sl], "KTP": KTp[sl], "VB": VBp[sl],
            "NRT": np.ascontiguousarray(NRTp[sl].transpose(1, 0, 2)),
            "NCT": np.ascontiguousarray(NCTp[sl].transpose(1, 0, 2)),
            "GS": gs,
        })
    res = run_bass_kernel_spmd(nc, in_maps, list(range(NCORES)))
    globals()["_last_results"] = res
    XOa = np.stack([res.results[c]["XO"] for c in range(NCORES)], axis=0)
    XOa = XOa.reshape(B * H, 128, NB, 64).astype(np.float32)
    X = XOa.transpose(0, 2, 1, 3).reshape(B * H, N, D)
    return np.ascontiguousarray(X).reshape(B, H, N, D)


# revision 29
# speedup vs baseline: 3.0193x; 1.0050x over previous
import numpy as np
import ml_dtypes

B, H, N, D = 4, 12, 8192, 64
M = 128
NCORES = 8
PAIRS = (B * H) // NCORES
NB = N // 128          # 64 N-blocks of 128 rows
NCH = N // 512         # 16 chunks of 512

_cache = {}


def _build():
    if "nc" in _cache:
        return _cache["nc"]
    import concourse.bacc as bacc
    import concourse.mybir as mybir
    import concourse.tile as tile

    f32 = mybir.dt.float32
    f16 = mybir.dt.float16
    bf16 = mybir.dt.bfloat16
    AF = mybir.ActivationFunctionType

    nc = bacc.Bacc("TRN2", target_bir_lowering=False, debug=False)
    QTP = nc.declare_dram_parameter("QTP", [PAIRS, 64, N], f16, isOutput=False)
    KTP = nc.declare_dram_parameter("KTP", [PAIRS, 64, N], f16, isOutput=False)
    VB = nc.declare_dram_parameter("VB", [PAIRS, 128, NB, 65], bf16, isOutput=False)
    NRT = nc.declare_dram_parameter("NRT", [64, PAIRS, 128], f16, isOutput=False)
    NCT = nc.declare_dram_parameter("NCT", [64, PAIRS, 128], f16, isOutput=False)
    GS = nc.declare_dram_parameter("GS", [1, 1], f32, isOutput=False)
    XO = nc.declare_dram_parameter("XO", [PAIRS, 128, NB, 64], bf16, isOutput=True)

    with tile.TileContext(nc) as tc:
        with (tc.tile_pool(name="pc", bufs=1) as pc,
              tc.tile_pool(name="pio", bufs=2) as pio,
              tc.tile_pool(name="pw", bufs=2) as pw,
              tc.tile_pool(name="pe3", bufs=3) as pe3,
              tc.tile_pool(name="psbig", bufs=2, space="PSUM") as psbig,
              tc.tile_pool(name="pss", bufs=1, space="PSUM") as pss):

            # ---- PSUM static regions ----
            xns = pss.tile([128, 2048], f32, tag="xns")
            SACC = xns[:, 0:512]          # S accumulate [:, 0:65]; X slots 0-3
            KVR = xns[:, 512:1280]        # NS kv/ps3/ps4; m-phase; A; X slots 4-9
            KVTR = xns[:, 1280:2048]      # NS kvt/ps5/ps6; X slots 10-15
            kvtr_bc = xns[:].bitcast(bf16)[:, 2560:4096]  # bf16 view of KVTR

            # ---- constants ----
            # tiny exp to pull the ACT table load off the critical path
            warm = pc.tile([1, 1], f32, tag="warm")
            nc.vector.memset(warm[:], 0.0)
            nc.scalar.activation(warm[:], warm[:], AF.Exp)

            ident = pc.tile([128, 128], bf16, tag="ident")
            nc.gpsimd.memset(ident[:], 0.0)
            nc.gpsimd.affine_select(out=ident[:], in_=ident[:],
                compare_op=mybir.AluOpType.not_equal, fill=1.0, base=0,
                pattern=[[-1, 128]], channel_multiplier=1)
            diag6 = {}
            for val in (7.0, 15.0, 13.0):
                t6 = pc.tile([128, PAIRS * 128], bf16, tag=f"i{int(val)}_6",
                             name=f"i{int(val)}_6")
                nc.gpsimd.memset(t6[:], 0.0)
                for q in range(PAIRS):
                    sl = t6[:, q * 128:(q + 1) * 128]
                    nc.gpsimd.affine_select(out=sl, in_=sl,
                        compare_op=mybir.AluOpType.not_equal, fill=val, base=0,
                        pattern=[[-1, 128]], channel_multiplier=1)
                diag6[val] = t6
            i7_6, i15_6, i13_6 = diag6[7.0], diag6[15.0], diag6[13.0]

            nrt_all = pc.tile([64, PAIRS, 128], f16, tag="nrt")
            nct_all = pc.tile([64, PAIRS, 128], f16, tag="nct")
            nc.sync.dma_start(nrt_all[:], NRT[:])
            nc.sync.dma_start(nct_all[:], NCT[:])
            ones_row = pc.tile([1, 128], f32, tag="ones_row")
            nc.vector.memset(ones_row[:], 1.0)
            gs_sb = pc.tile([1, 1], f32, tag="gs_sb")
            nc.sync.dma_start(gs_sb[:], GS[:])

            ps_gs = xns[:, 448:449]
            nc.tensor.matmul(ps_gs, ones_row[:], gs_sb[:], start=True, stop=True)
            gsb = pc.tile([128, 1], f32, tag="gsb")
            nc.vector.tensor_copy(gsb[:], ps_gs)

            ball = pc.tile([128, PAIRS, 65], bf16, tag="ball")
            s_bf6 = pc.tile([128, PAIRS, 64], bf16, tag="s_bf6")

            # ---- m / k2 prep for ALL pairs: matmuls up front, the rest is
            # injected into the stream at slot 1 so the first er logits are
            # not queued behind the transposes.
            e_m6 = pc.tile([128, PAIRS * 128], f32, tag="e_m6")
            msum6 = pc.tile([128, PAIRS], f32, tag="msum6")
            k2t6 = pc.tile([128, PAIRS * 128], bf16, tag="k2t6")
            k2_6 = pc.tile([128, PAIRS * 128], bf16, tag="k2_6")
            for q in range(PAIRS):
                nc.tensor.matmul(KVR[:, q * 128:(q + 1) * 128],
                                 nrt_all[:, q, :], nct_all[:, q, :],
                                 start=True, stop=True)
            ns_state = {}

            # ACT/DVE part of the m-phase runs before the stream (fills the
            # first kt DMA wait); the PE transposes are injected at slot 1.
            for q in range(PAIRS):
                nc.scalar.activation(e_m6[:, q * 128:(q + 1) * 128],
                                     KVR[:, q * 128:(q + 1) * 128],
                                     AF.Exp, accum_out=msum6[:, q:q + 1])
            mrec6 = pc.tile([128, PAIRS, 1], f32, tag="mrec6")
            nc.vector.reciprocal(mrec6[:], msum6[:].rearrange(
                "p (q o) -> p q o", o=1))
            nc.vector.tensor_tensor(
                out=k2_6[:].rearrange("p (q e) -> p q e", e=128),
                in0=e_m6[:].rearrange("p (q e) -> p q e", e=128),
                in1=mrec6[:].to_broadcast([128, PAIRS, 128]),
                op=mybir.AluOpType.mult)

            def m_phase_rest():
                for q in range(PAIRS):
                    nc.tensor.transpose(kvtr_bc[:, q * 256:q * 256 + 128],
                                        k2_6[:, q * 128:(q + 1) * 128],
                                        ident[:])
                kvtr_slots = kvtr_bc.rearrange(
                    "p (q e) -> p q e", e=256)[:, :, 0:128]
                nc.vector.tensor_copy(
                    k2t6[:].rearrange("p (q e) -> p q e", e=128), kvtr_slots)
                vm6 = pw.tile([128, PAIRS * 128], bf16, tag="vm6")
                nc.vector.tensor_scalar_mul(
                    vm6[:].rearrange("p (q e) -> p q e", e=128), kvtr_slots,
                    gsb[:])
                vmt6 = pw.tile([128, PAIRS * 128], bf16, tag="vmt6")
                nc.vector.tensor_scalar_mul(vmt6[:], k2_6[:], gsb[:])
                ns_state.update({"vm6": vm6, "vmt6": vmt6})

            # ---- Newton-Schulz stages (one iteration = 4 stages, paced one
            # stage per two er-steps so the PE never queues behind a pending
            # DVE op; each stage's PE matmuls depend only on DVE work from
            # >=2 er-steps earlier) ----

            def ns_stage(j, stage):
                vm6, vmt6 = ns_state["vm6"], ns_state["vmt6"]
                if stage == 0:
                    for q in range(PAIRS):
                        nc.tensor.matmul(KVR[:, q * 128:(q + 1) * 128],
                                         k2t6[:, q * 128:(q + 1) * 128],
                                         vm6[:, q * 128:(q + 1) * 128],
                                         start=True, stop=True)
                    for q in range(PAIRS):
                        nc.tensor.matmul(KVTR[:, q * 128:(q + 1) * 128],
                                         vm6[:, q * 128:(q + 1) * 128],
                                         k2t6[:, q * 128:(q + 1) * 128],
                                         start=True, stop=True)
                    kvt6 = pw.tile([128, PAIRS * 128], bf16, tag="kvt6",
                                   name="kvt6")
                    nc.vector.tensor_copy(kvt6[:], KVTR)
                    u1 = pw.tile([128, PAIRS * 128], bf16, tag="u1", name="u1")
                    nc.vector.tensor_sub(u1[:], i7_6[:], KVR)
                    ns_state["kvt6"], ns_state["u1"] = kvt6, u1
                elif stage == 1:
                    kvt6, u1 = ns_state["kvt6"], ns_state["u1"]
                    for q in range(PAIRS):
                        nc.tensor.matmul(KVR[:, q * 128:(q + 1) * 128],
                                         kvt6[:, q * 128:(q + 1) * 128],
                                         u1[:, q * 128:(q + 1) * 128],
                                         start=True, stop=True)
                    u3 = pw.tile([128, PAIRS * 128], bf16, tag="u3", name="u3")
                    nc.vector.tensor_sub(u3[:], i15_6[:], KVR)
                    ns_state["u3"] = u3
                elif stage == 2:
                    kvt6, u3 = ns_state["kvt6"], ns_state["u3"]
                    for q in range(PAIRS):
                        nc.tensor.matmul(KVR[:, q * 128:(q + 1) * 128],
                                         kvt6[:, q * 128:(q + 1) * 128],
                                         u3[:, q * 128:(q + 1) * 128],
                                         start=True, stop=True)
                    u5 = pw.tile([128, PAIRS * 128], bf16, tag="u5", name="u5")
                    nc.vector.tensor_sub(u5[:], i13_6[:], KVR)
                    ns_state["u5"] = u5
                elif stage == 3:
                    # ps5 -> KVTR, ps6 -> KVR: the two scale ops read
                    # different regions, so neither matmul set waits on the
                    # other's DVE op.
                    u5 = ns_state["u5"]
                    if j < 5:
                        for q in range(PAIRS):
                            nc.tensor.matmul(KVTR[:, q * 128:(q + 1) * 128],
                                             vmt6[:, q * 128:(q + 1) * 128],
                                             u5[:, q * 128:(q + 1) * 128],
                                             start=True, stop=True)
                    for q in range(PAIRS):
                        nc.tensor.matmul(KVR[:, q * 128:(q + 1) * 128],
                                         u5[:, q * 128:(q + 1) * 128],
                                         vmt6[:, q * 128:(q + 1) * 128],
                                         start=True, stop=True)
                    if j < 5:
                        vm6n = pw.tile([128, PAIRS * 128], bf16, tag="vm6",
                                       name="vm6")
                        nc.vector.tensor_scalar(vm6n[:], KVTR, 0.25,
                                                scalar2=None,
                                                op0=mybir.AluOpType.mult)
                        ns_state["vm6"] = vm6n
                    vmt6n = pw.tile([128, PAIRS * 128], bf16, tag="vmt6",
                                    name="vmt6")
                    nc.vector.tensor_scalar(vmt6n[:], KVR, 0.25, scalar2=None,
                                            op0=mybir.AluOpType.mult)
                    ns_state["vmt6"] = vmt6n

            # ---------------- sweep 1: er logits -> exp -> S ----------------
            kt_t = [None] * PAIRS
            vb_t = [None] * PAIRS
            qt_t = [None] * PAIRS

            def load_kv(p):
                kt_t[p] = pio.tile([64, N], f16, tag="kt", name="kt")
                nc.sync.dma_start(kt_t[p][:, 0:N // 2], KTP[p][:, 0:N // 2])
                vb_t[p] = pio.tile([128, NB, 65], bf16, tag="vb", name="vb")
                nc.sync.dma_start(vb_t[p][:, 0:NB // 2, :],
                                  VB[p][:, 0:NB // 2, :])
                nc.sync.dma_start(kt_t[p][:, N // 2:N], KTP[p][:, N // 2:N])
                nc.sync.dma_start(vb_t[p][:, NB // 2:NB, :],
                                  VB[p][:, NB // 2:NB, :])

            def load_q(p):
                qt_t[p] = pio.tile([64, N], f16, tag="qt", name="qt", bufs=3)
                nc.sync.dma_start(qt_t[p][:], QTP[p])

            load_kv(0)
            load_kv(1)

            # Global batch stream: producer (logits+exp) leads the consumer
            # (S matmuls) by 2 batches, so the PE always has the next pair's
            # logit matmuls queued ahead of dependency-stalled consumers.
            NBAT = PAIRS * 8
            erc_all = [None] * NBAT
            ns_emitted = 0
            for g in range(NBAT + 2):
                if g < NBAT:
                    p, k = divmod(g, 8)
                    if k == 0 and p + 2 < PAIRS:
                        load_kv(p + 2)
                    if k == 0 and p >= 3:
                        load_q(p - 3)
                    nr = nrt_all[:, p, :]
                    kt = kt_t[p]
                    ps_er = psbig.tile([128, 1024], f32, tag="big",
                                       name="ps_er")
                    for j in range(8):
                        blk = 8 * k + j
                        nc.tensor.matmul(
                            ps_er[:, j * 128:(j + 1) * 128],
                            kt[:, blk * 128:(blk + 1) * 128],
                            nr, start=True, stop=True)
                    erc_all[g] = pe3.tile([128, 1024], bf16, tag="erc",
                                          name="erc", bufs=8)
                    nc.scalar.activation(erc_all[g][:], ps_er[:], AF.Exp)
                if g >= 2:
                    gc = g - 2
                    pc_, kc = divmod(gc, 8)
                    erc = erc_all[gc]
                    vb = vb_t[pc_]
                    soff = 128 * (pc_ % 2)
                    for s in range(8):
                        t = 8 * kc + s
                        nc.tensor.matmul(
                            SACC[:, soff:soff + 65],
                            erc[:, s * 128:(s + 1) * 128],
                            vb[:, t, :],
                            start=(kc == 0 and s == 0),
                            stop=(kc == 7 and s == 7),
                            skip_group_check=True)
                    erc_all[gc] = None
                    if kc == 7:
                        srec = pw.tile([128, 1], f32, tag="srec")
                        nc.vector.reciprocal(
                            srec[:], SACC[:, soff + 64:soff + 65])
                        nc.vector.tensor_scalar_mul(
                            s_bf6[:, pc_, :], SACC[:, soff:soff + 64],
                            srec[:])
                        kt_t[pc_] = None
                        vb_t[pc_] = None
                if g == 1:
                    m_phase_rest()
                if g >= 3 and (g - 3) % 2 == 0 and ns_emitted < 24:
                    ns_stage(ns_emitted // 4, ns_emitted % 4)
                    ns_emitted += 1

            # ---- A = Vm_final @ S_norm; b = [A | 1] ----
            vmt6 = ns_state["vmt6"]
            for q in range(PAIRS):
                nc.tensor.matmul(KVR[:, q * 128:q * 128 + 64],
                                 vmt6[:, q * 128:(q + 1) * 128],
                                 s_bf6[:, q, :], start=True, stop=True)
            nc.vector.tensor_copy(
                ball[:, :, 0:64],
                KVR.rearrange("p (q e) -> p q e", e=128)[:, :, 0:64])
            nc.vector.memset(ball[:, :, 64:65], 1.0)

            # ---------------- sweep 2: ec logits -> exp -> X ----------------
            # Same global-stream structure, consumer lag 2; output DMA is
            # chunked per division group so the store overlaps compute.
            xv = xns[:].rearrange("p (s e) -> p s e", e=128)
            xsb_t = [None] * PAIRS
            ec_all = [None] * NBAT
            for g in range(NBAT + 2):
                if g < NBAT:
                    p, i = divmod(g, 8)
                    if i == 0 and p + 3 < PAIRS:
                        load_q(p + 3)
                    if i == 0:
                        xsb_t[p] = pio.tile([128, NB, 64], bf16, tag="xsb",
                                            name="xsb")
                    qt = qt_t[p]
                    ncs = nct_all[:, p, :]
                    ps_ec = psbig.tile([128, 1024], f32, tag="big",
                                       name="ps_ec")
                    nc.tensor.matmul(ps_ec[:, 0:512], ncs,
                                     qt[:, (2 * i) * 512:(2 * i + 1) * 512],
                                     start=True, stop=True)
                    nc.tensor.matmul(ps_ec[:, 512:1024], ncs,
                                     qt[:, (2 * i + 1) * 512:(2 * i + 2) * 512],
                                     start=True, stop=True)
                    ec_all[g] = pe3.tile([128, 1024], bf16, tag="ec",
                                         name="ec", bufs=4)
                    nc.scalar.activation(ec_all[g][:], ps_ec[:], AF.Exp)
                if g >= 2:
                    gc = g - 2
                    pc_, ic = divmod(gc, 8)
                    ec_sb = ec_all[gc]
                    b_p = ball[:, pc_, :]
                    xsb = xsb_t[pc_]
                    for c in range(8):
                        sl = (ic % 2) * 8 + c
                        nc.tensor.matmul(
                            xns[:, sl * 128:sl * 128 + 65],
                            ec_sb[:, c * 128:(c + 1) * 128], b_p,
                            start=True, stop=True)
                    ec_all[gc] = None
                    if ic % 2 == 1:
                        m2 = ic // 2
                        rec = pw.tile([128, 16, 1], f32, tag="rec")
                        nc.vector.reciprocal(rec[:], xv[:, :, 64:65])
                        nc.vector.tensor_tensor(
                            out=xsb[:, 16 * m2:16 * (m2 + 1), :],
                            in0=xv[:, :, 0:64],
                            in1=rec[:].to_broadcast([128, 16, 64]),
                            op=mybir.AluOpType.mult)
                        nc.sync.dma_start(
                            XO[pc_][:, 16 * m2:16 * (m2 + 1), :],
                            xsb[:, 16 * m2:16 * (m2 + 1), :])
                    if ic == 7:
                        qt_t[pc_] = None
    nc.finalize()
    _cache["nc"] = nc
    return nc


def kernel(Q, K, V, mask):
    from concourse.bass_utils import run_bass_kernel_spmd

    Q = np.asarray(Q, dtype=np.float32)
    K = np.asarray(K, dtype=np.float32)
    V = np.asarray(V, dtype=np.float32)
    Qf = Q.reshape(B * H, N, D)
    Kf = K.reshape(B * H, N, D)
    Vf = V.reshape(B * H, N, D)

    nct = np.empty((B * H, D, M), np.float32)
    nrt = np.empty((B * H, D, M), np.float32)
    gmax = 0.0
    for i in range(B * H):
        for (T, out) in ((Kf, nct), (Qf, nrt)):
            s = T[i, :, 0].copy()
            s[0] = np.inf
            idx = np.argpartition(-s, M)[:M]
            out[i] = T[i, np.sort(idx), :].T
        nr = nrt[i].T.astype(np.float64)
        nc_ = nct[i].T.astype(np.float64)
        m = nr @ nc_.T
        e = np.exp(m - m.max(axis=1, keepdims=True))
        k2 = e / e.sum(axis=1, keepdims=True)
        gmax = max(gmax, float(k2.sum(axis=0).max()))

    QTp = np.ascontiguousarray(Qf.transpose(0, 2, 1)).astype(np.float16)
    KTp = np.ascontiguousarray(Kf.transpose(0, 2, 1)).astype(np.float16)

    Vext = np.concatenate([Vf, np.ones((B * H, N, 1), np.float32)], axis=2)
    VBp = np.ascontiguousarray(
        Vext.reshape(B * H, NB, 128, 65).transpose(0, 2, 1, 3)
    ).astype(ml_dtypes.bfloat16)

    NRTp = nrt.astype(np.float16)   # [48, 64, 128]
    NCTp = nct.astype(np.float16)
    gs = np.array([[1.0 / gmax]], np.float32)

    nc = _build()
    in_maps = []
    for c in range(NCORES):
        sl = slice(c * PAIRS, (c + 1) * PAIRS)
        in_maps.append({
            "QTP": QTp[sl], "KTP": KTp[sl], "VB": VBp[sl],
            "NRT": np.ascontiguousarray(NRTp[sl].transpose(1, 0, 2)),
            "NCT": np.ascontiguousarray(NCTp[sl].transpose(1, 0, 2)),
            "GS": gs,
        })
    res = run_bass_kernel_spmd(nc, in_maps, list(range(NCORES)))
    globals()["_last_results"] = res
    XOa = np.stack([res.results[c]["XO"] for c in range(NCORES)], axis=0)
    XOa = XOa.reshape(B * H, 128, NB, 64).astype(np.float32)
    X = XOa.transpose(0, 2, 1, 3).reshape(B * H, N, D)
    return np.ascontiguousarray(X).reshape(B, H, N, D)
